# revision 1
# baseline (speedup 1.0000x reference)
"""Sliding-window causal self-attention on 8 trn2 NeuronCores.

Problem: B=2, T=4096, C=512, H=8 heads (d=64), window MEMORY=256
    qkv = x @ w_attn.T ; per-head windowed-causal softmax attention ; y @ w_proj.T

Sharding: sequence-parallel. B*T = 8192 rows -> 8 chunks of 1024 queries
(4 chunks per batch element). Each core receives its 1024 query rows plus a
256-row halo of preceding tokens (zero-padded at batch starts) and computes
its output slice independently -- no collectives. The host pre-transposes
x/w so no on-chip transposes are needed anywhere.

Device dataflow (per core):
  1) QKV: Q,K head-major [d, t] (a pair of heads shares each 128-partition
     tile); V token-major [t, (h, d|ones)] with a 64-wide ones block per head.
  2) Scores: S^T[j,i] = kT.T @ qT per (head, key-block jb), both heads of a
     pair packed via tile_position row groups. Each query block attends to
     exactly 3 key blocks; one 384-wide matmul per (head, jb) covers all its
     consumers, and the in-band mask is linear: valid iff 0 <= c - b <= 256.
  3) P = exp(S/8 + kbias) via one ACT op per (pair, jb); kbias is the
     per-key (= per-partition) -inf bias masking the zero-padded halo.
     The 0/1 band mask multiplies only the two triangular 128-col blocks
     (the middle block is all-ones).
  4) yT_aug = [V_h | ones].T @ P accumulated over key blocks: psum partitions
     0:64 hold the unnormalized output, 64:128 the softmax denominator
     replicated by the matmul itself. start=True rides the first sub-matmul
     per psum bank (whole-bank has_written clear). Normalization is one DVE
     reciprocal (base-shift 64->0) + one psum*sbuf multiply per head.
  5) Projection: out[t, C] = yT.T @ wpT over 4 c-tiles; evictions on ACT.

Dtypes: x, w_attn shipped bf16 (halves the startup DMA, which gates the
first matmuls); Q/K/P/V bf16; S/yT/w_proj float32r (full-rate reduced-
precision fp32); all psum accumulation fp32. End-to-end rel err ~3.3e-3.

Timing note: engines execute their instruction streams in order, so S(jb+1)
is emitted before exp/mask/AV(jb) (2-stage software pipeline), and the
normalization is interleaved at the last key block of each psum bank.
"""

import numpy as np
import ml_dtypes

import concourse.mybir as mybir
import concourse.tile as tile
from concourse import bacc
from concourse.bass_utils import run_bass_kernel_spmd

B, T, C = 2, 4096, 512
H, D = 8, 64
MEM = 256
NCORES = 8
TQ = 1024            # queries per core
TL = TQ + MEM        # local tokens incl halo = 1280
NQB = TQ // 128      # 8 query blocks
NJB = TL // 128      # 10 key blocks
NPAIR = 4            # head pairs
KT = C // 128        # 4 contraction tiles
F32 = mybir.dt.float32
F32R = mybir.dt.float32r
BF16 = mybir.dt.bfloat16
MASKVAL = -30000.0

_cache = {}


def _consumers(jb):
    """Query blocks consuming key block jb, and the band-mask column offset."""
    gmin = max(0, jb - 2)
    gmax = min(NQB - 1, jb)
    coff = (gmin - (jb - 2)) * 128
    return gmin, gmax, coff


def _build(loop_iters=0, stages=3, skip_norm=False, skip_mask=False, skip_exp=False):
    nc = bacc.Bacc(None, target_bir_lowering=False, name="swattn")

    xT = nc.dram_tensor("xT", [C, TL], BF16, kind="ExternalInput")
    wqkT = nc.dram_tensor("wqkT", [C, 3 * C], BF16, kind="ExternalInput")
    wpT = nc.dram_tensor("wpT", [C, C], F32R, kind="ExternalInput")
    kb = nc.dram_tensor("kb", [128, NJB], F32, kind="ExternalInput")
    mask = nc.dram_tensor("mask", [128, 2, 384], BF16, kind="ExternalInput")
    y = nc.dram_tensor("y", [TQ, C], F32, kind="ExternalOutput")
    with tile.TileContext(nc) as tc:
        with tc.tile_pool(name="persist", bufs=1) as pers:
            import contextlib
            loop = tc.For_i(0, loop_iters, 1) if loop_iters else contextlib.nullcontext()
            with loop:
                kb_sb = pers.tile([128, NJB], F32)
                mask_sb = pers.tile([128, 2, 384], BF16)
                # per-k-tile input tiles + chunked DMAs so the first matmuls start
                # early (one monolithic DMA stalled the PE ~19us at kernel start)
                xT_r = xT[:].rearrange("(ko ki) t -> ki ko t", ki=128)
                wqk_r = wqkT[:].rearrange("(ko ki) f -> ki ko f", ki=128)
                wp_r = wpT[:].rearrange("(ko ki) f -> ki ko f", ki=128)
                xT_k = [pers.tile([128, TL], BF16, name=f"xT{k}") for k in range(KT)]
                wqk_k = [pers.tile([128, 3 * C], BF16, name=f"wqk{k}") for k in range(KT)]
                wp_k = [pers.tile([128, C], F32R, name=f"wp{k}") for k in range(KT)]
                # x chunks on the sync HWDGE queue, weights on the scalar HWDGE
                # queue -- two descriptor generators run in parallel, and each
                # QKV group unblocks as soon as its k-chunks land.
                if stages >= 1:
                    for k in range(KT):
                        nc.sync.dma_start(xT_k[k][:, :640], xT_r[:, k, :640])
                        nc.scalar.dma_start(
                            wqk_k[k][:, : 2 * C], wqk_r[:, k, : 2 * C]
                        )
                        nc.sync.dma_start(xT_k[k][:, 640:], xT_r[:, k, 640:])
                        nc.scalar.dma_start(
                            wqk_k[k][:, 2 * C :], wqk_r[:, k, 2 * C :]
                        )
                    for k in range(KT):
                        nc.scalar.dma_start(wp_k[k][:], wp_r[:, k, :])
                    nc.sync.dma_start(kb_sb[:], kb[:])
                    nc.sync.dma_start(mask_sb[:], mask[:])

                # Q,K head-major [d, t]; pair p: partitions 0:64 = head 2p, 64:128 = head 2p+1
                qT_sb = pers.tile([128, NPAIR, TQ], BF16)
                kT_sb = pers.tile([128, NPAIR, TL], BF16)
                # V token-major, padded with a 64-wide ones block per head:
                # AV matmuls with lhsT=[V_h | ones] then write yT_un on psum
                # partitions 0:64 and the replicated softmax denominator on
                # partitions 64:128 -- broadcast comes free from the matmul.
                v_sb = pers.tile([128, NJB, H, 128], BF16)
                nc.gpsimd.memset(v_sb[:, :, :, D:], 1.0)
                # normalized attention output, c-major [c, t]
                yt_sb = pers.tile([128, KT, TQ], F32R)

                if stages < 1:
                    dummy = pers.tile([128, 64], F32, name="dummy")
                    nc.vector.memset(dummy[:], 1.0)
                # ---- stage 1: QKV projection ----
                # ps_a spans stages 1 and 3: the projection reuses the QKV psum
                # banks (same tag), so it never WAR-blocks on attention psum.
                with tc.tile_pool(name="ps_a", bufs=2, space="PSUM") as ps_qkv:
                  if stages >= 1:
                    # Q: own queries only (local tokens 256:1280); two
                    # 512-wide matmul groups share one 2-bank psum tile so a
                    # single wide eviction replaces two.
                    for p in range(NPAIR):
                        pq = ps_qkv.tile([128, TQ], F32, tag="qkv2", name=f"pq{p}")
                        for t0 in range(0, TQ, 512):
                            for k in range(KT):
                                nc.tensor.matmul(
                                    pq[:, t0 : t0 + 512],
                                    wqk_k[k][:, p * 128 : (p + 1) * 128],
                                    xT_k[k][:, MEM + t0 : MEM + t0 + 512],
                                    start=(k == 0), stop=(k == KT - 1),
                                )
                        nc.scalar.copy(qT_sb[:, p, :], pq[:])
                    # K: all local tokens; 1024-wide tile + 256 remainder
                    for p in range(NPAIR):
                        pk = ps_qkv.tile([128, TQ], F32, tag="qkv2", name=f"pk{p}")
                        for t0 in (0, 512):
                            for k in range(KT):
                                nc.tensor.matmul(
                                    pk[:, t0 : t0 + 512],
                                    wqk_k[k][:, C + p * 128 : C + (p + 1) * 128],
                                    xT_k[k][:, t0 : t0 + 512],
                                    start=(k == 0), stop=(k == KT - 1),
                                )
                        nc.vector.tensor_copy(kT_sb[:, p, 0:TQ], pk[:])
                        pk2 = ps_qkv.tile([128, 256], F32, tag="qkv", name=f"pk2{p}")
                        for k in range(KT):
                            nc.tensor.matmul(
                                pk2[:],
                                wqk_k[k][:, C + p * 128 : C + (p + 1) * 128],
                                xT_k[k][:, TQ : TQ + 256],
                                start=(k == 0), stop=(k == KT - 1),
                            )
                        nc.vector.tensor_copy(kT_sb[:, p, TQ:], pk2[:])
                    # V: token-major, two token-blocks per psum tile
                    for tb in range(0, NJB, 2):
                        pv = ps_qkv.tile([128, TQ], F32, tag="qkv2", name=f"pv{tb}")
                        for sub in range(2):
                            for k in range(KT):
                                nc.tensor.matmul(
                                    pv[:, sub * 512 : (sub + 1) * 512],
                                    xT_k[k][:, (tb + sub) * 128 : (tb + sub + 1) * 128],
                                    wqk_k[k][:, 2 * C : 3 * C],
                                    start=(k == 0), stop=(k == KT - 1),
                                )
                        nc.scalar.copy(
                            v_sb[:, tb : tb + 2, :, 0:D],
                            pv[:].rearrange("t (b h d) -> t b h d", b=2, h=H),
                        )

                # ---- stage 2: attention, one head pair at a time ----
                if stages < 2:
                    pass
                elif True:
                  with (
                    tc.tile_pool(name="ps_s", bufs=2, space="PSUM") as ps_s,
                    tc.tile_pool(name="ps_y", bufs=2, space="PSUM") as ps_y,
                    tc.tile_pool(name="ptile", bufs=4) as ppool,
                    tc.tile_pool(name="norm", bufs=3) as npool,
                ):
                    for p in range(NPAIR):
                        yps = [
                            ps_y.tile([128, TQ], F32, tag="yt", name=f"yt{p}_{i}")
                            for i in range(2)
                        ]
                        def emit_s(jb):
                            gmin, gmax, coff = _consumers(jb)
                            ncols = (gmax - gmin + 1) * 128
                            s_ps = ps_s.tile([128, 2, 512], F32, tag="s", name=f"s{p}_{jb}")
                            for hh in range(2):
                                nc.tensor.matmul(
                                    s_ps[:, hh, :ncols],
                                    kT_sb[hh * 64 : hh * 64 + 64, p, jb * 128 : (jb + 1) * 128],
                                    qT_sb[hh * 64 : hh * 64 + 64, p, gmin * 128 : (gmax + 1) * 128],
                                    start=True, stop=True,
                                )
                            return s_ps

                        def emit_rest(jb, s_ps):
                            gmin, gmax, coff = _consumers(jb)
                            ncols = (gmax - gmin + 1) * 128
                            p_sb = ppool.tile([128, 2, 384], BF16, tag="p", name=f"p{p}_{jb}")
                            if skip_exp:
                                nc.vector.tensor_copy(p_sb[:, :, :ncols], s_ps[:, :, :ncols])
                            else:
                                nc.scalar.activation(
                                    p_sb[:, :, :ncols],
                                    s_ps[:, :, :ncols],
                                    mybir.ActivationFunctionType.Exp,
                                    bias=kb_sb[:, jb : jb + 1],
                                    scale=0.125,
                                )
                            # only the two triangular 128-col blocks of the
                            # band need masking; the middle block is all-ones
                            mranges = [
                                r0 for r0 in range(0, ncols, 128)
                                if coff + r0 in (0, 256)
                            ]
                            if skip_mask:
                                mranges = []
                            if mranges == [0, 256]:
                                # one strided op covering both triangle blocks
                                nc.vector.tensor_tensor(
                                    p_sb[:, :, :].rearrange(
                                        "p h (r c) -> p h r c", c=128
                                    )[:, :, 0:3:2],
                                    p_sb[:, :, :].rearrange(
                                        "p h (r c) -> p h r c", c=128
                                    )[:, :, 0:3:2],
                                    mask_sb[:, :, :].rearrange(
                                        "p h (r c) -> p h r c", c=128
                                    )[:, :, 0:3:2],
                                    mybir.AluOpType.mult,
                                )
                            else:
                                for r0 in mranges:
                                    nc.vector.tensor_tensor(
                                        p_sb[:, :, r0 : r0 + 128],
                                        p_sb[:, :, r0 : r0 + 128],
                                        mask_sb[:, :, coff + r0 : coff + r0 + 128],
                                        mybir.AluOpType.mult,
                                    )

                            # AV: one wide matmul per (head, key-block), split at
                            # the 512-col PSUM bank boundary. All start=False --
                            # the banks were zero-cleared by the K=1 matmuls above
                            # (start=True clears has_written for the WHOLE bank,
                            # so per-column-group starts are unusable).
                            c0 = gmin * 128
                            c1 = (gmax + 1) * 128
                            for hh in range(2):
                                h = 2 * p + hh
                                for a, b in ((c0, min(c1, 512)), (max(c0, 512), c1)):
                                    if a >= b:
                                        continue
                                    # start=True exactly on the first
                                    # sub-matmul touching each 512-col bank:
                                    # it clears the whole bank's has_written,
                                    # so later matmuls overwrite-and-set
                                    # fresh columns and accumulate written
                                    # ones -- no explicit zero-fill needed.
                                    nc.tensor.matmul(
                                        yps[hh][:, a:b],
                                        v_sb[:, jb, h, :],
                                        p_sb[:, hh, a - c0 : b - c0],
                                        start=(jb == 0 and a == 0)
                                        or (jb == 4 and a == 512),
                                        stop=(jb == NJB - 1 and b == c1),
                                        skip_group_check=True,
                                    )

                            # normalization once per head after the last key-block:
                            # denominator is replicated on psum partitions 64:128, so
                            # it is one DVE reciprocal (base-shift 64->0) and one
                            # psum*sbuf multiply over the full 1024 columns.
                            if jb == NJB - 1 and not skip_norm:
                                with nc.allow_low_precision(
                                    reason="softmax weights are O(1); bf16 out is ample"
                                ):
                                    for hh in range(2):
                                        rec = npool.tile([64, TQ], F32, tag="rec")
                                        nc.vector.reciprocal(rec[:], yps[hh][64:128, :])
                                        nc.vector.tensor_tensor(
                                            yt_sb[hh * 64 : hh * 64 + 64, p, :],
                                            yps[hh][0:64, :],
                                            rec[:],
                                            mybir.AluOpType.mult,
                                        )

                    # 2-stage software pipeline: the PE stream must carry
                        # S(jb+1) BEFORE AV(jb), since engines execute their
                        # streams strictly in order -- otherwise AV(jb) stalling
                        # on exp/mask(jb) blocks the already-ready S(jb+1).
                        pending = None
                        for jb in range(NJB):
                            sp = emit_s(jb)
                            if pending is not None:
                                emit_rest(pending[0], pending[1])
                            pending = (jb, sp)
                        emit_rest(pending[0], pending[1])

                # ---- stage 3: output projection ----
                if stages < 3:
                    pass
                elif True:
                  with (
                    tc.tile_pool(name="ps_o", bufs=4, space="PSUM") as ps_o,
                    tc.tile_pool(name="obuf", bufs=4) as opool,
                ):
                    for g in range(NQB):
                        po = ps_o.tile([128, C], F32, tag="o")
                        for k in range(KT):
                            nc.tensor.matmul(
                                po[:],
                                yt_sb[:, k, g * 128 : (g + 1) * 128],
                                wp_k[k][:],
                                start=(k == 0), stop=(k == KT - 1),
                            )
                        o_sb = opool.tile([128, C], F32, tag="ob")
                        nc.scalar.copy(o_sb[:], po[:])
                        nc.sync.dma_start(y[g * 128 : (g + 1) * 128, :], o_sb[:])

    nc.finalize()
    return nc


def _host_inputs(x, w_attn, w_proj):
    """Build per-core input maps (numpy only)."""
    wqkT = np.ascontiguousarray(w_attn.T.astype(ml_dtypes.bfloat16))
    wpT = np.ascontiguousarray(w_proj.T.astype(np.float32))

    # band mask [128, 384]: valid iff 0 <= c - b <= MEM
    b = np.arange(128)[:, None]
    c = np.arange(384)[None, :]
    mask = ((c - b >= 0) & (c - b <= MEM)).astype(ml_dtypes.bfloat16)
    mask = np.ascontiguousarray(np.broadcast_to(mask[:, None, :], (128, 2, 384)))

    in_maps = []
    for core in range(NCORES):
        bi, ci = divmod(core, T // TQ)
        q0 = ci * TQ
        x_loc = np.zeros((TL, C), dtype=np.float32)
        lo = q0 - MEM
        src0 = max(0, lo)
        x_loc[src0 - lo :] = x[bi, src0 : q0 + TQ]
        xT_loc = np.ascontiguousarray(x_loc.T.astype(ml_dtypes.bfloat16))

        kb = np.zeros((128, NJB), dtype=np.float32)
        if lo < 0:
            pad = -lo  # number of padded (invalid) leading keys
            for jb in range(NJB):
                k0 = jb * 128
                if k0 >= pad:
                    break
                kb[: min(128, pad - k0), jb] = MASKVAL

        in_maps.append(
            {"xT": xT_loc, "wqkT": wqkT, "wpT": wpT, "kb": kb, "mask": mask}
        )
    return in_maps


def kernel(x, w_attn, w_proj):
    x = np.asarray(x, dtype=np.float32)
    w_attn = np.asarray(w_attn, dtype=np.float32)
    w_proj = np.asarray(w_proj, dtype=np.float32)

    if "nc" not in _cache:
        _cache["nc"] = _build()
    nc = _cache["nc"]

    in_maps = _host_inputs(x, w_attn, w_proj)
    res = run_bass_kernel_spmd(nc, in_maps, core_ids=list(range(NCORES)))

    out = np.empty((B, T, C), dtype=np.float32)
    for core in range(NCORES):
        bi, ci = divmod(core, T // TQ)
        out[bi, ci * TQ : (ci + 1) * TQ] = res.results[core]["y"]
    return out



# revision 6
# speedup vs baseline: 1.8725x; 1.8725x over previous
"""Sliding-window causal self-attention on 8 trn2 NeuronCores.

Problem: B=2, T=4096, C=512, H=8 heads (d=64), window MEMORY=256
    qkv = x @ w_attn.T ; per-head windowed-causal softmax attention ; y @ w_proj.T

Sharding: sequence-parallel. B*T = 8192 rows -> 8 chunks of 1024 queries
(4 chunks per batch element). Each core receives its 1024 query rows plus a
256-row halo of preceding tokens (zero-padded at batch starts) and computes
its output slice independently -- no collectives. The host pre-transposes
x/w so no on-chip transposes are needed anywhere.

Device dataflow (per core):
  1) QKV: Q,K head-major [d, t] (a pair of heads shares each 128-partition
     tile); V token-major [t, (h, d|ones)] with a 64-wide ones block per head.
  2) Scores: S^T[j,i] = kT.T @ qT per (head, key-block jb), both heads of a
     pair packed via tile_position row groups. Each query block attends to
     exactly 3 key blocks; one 384-wide matmul per (head, jb) covers all its
     consumers, and the in-band mask is linear: valid iff 0 <= c - b <= 256.
  3) P = exp(S/8 + kbias) via one ACT op per (pair, jb); kbias is the
     per-key (= per-partition) -inf bias masking the zero-padded halo.
     The 0/1 band mask multiplies only the two triangular 128-col blocks
     (the middle block is all-ones).
  4) yT_aug = [V_h | ones].T @ P accumulated over key blocks: psum partitions
     0:64 hold the unnormalized output, 64:128 the softmax denominator
     replicated by the matmul itself. start=True rides the first sub-matmul
     per psum bank (whole-bank has_written clear). Normalization is one DVE
     reciprocal (base-shift 64->0) + one psum*sbuf multiply per head.
  5) Projection: out[t, C] = yT.T @ wpT over 4 c-tiles; evictions on ACT.

Dtypes: x, w_attn shipped bf16 (halves the startup DMA, which gates the
first matmuls); Q/K/P/V bf16; S/yT/w_proj float32r (full-rate reduced-
precision fp32); all psum accumulation fp32. End-to-end rel err ~3.3e-3.

Timing note: engines execute their instruction streams in order, so S(jb+1)
is emitted before exp/mask/AV(jb) (2-stage software pipeline), and the
normalization is interleaved at the last key block of each psum bank.
"""

import numpy as np
import ml_dtypes

import concourse.mybir as mybir
import concourse.tile as tile
from concourse import bacc
from concourse.bass_utils import run_bass_kernel_spmd

B, T, C = 2, 4096, 512
H, D = 8, 64
MEM = 256
NCORES = 8
TQ = 1024            # queries per core
TL = TQ + MEM        # local tokens incl halo = 1280
NQB = TQ // 128      # 8 query blocks
NJB = TL // 128      # 10 key blocks
NPAIR = 4            # head pairs
KT = C // 128        # 4 contraction tiles
F32 = mybir.dt.float32
F32R = mybir.dt.float32r
BF16 = mybir.dt.bfloat16
MASKVAL = -30000.0

_cache = {}


def _consumers(jb):
    """Query blocks consuming key block jb, and the band-mask column offset."""
    gmin = max(0, jb - 2)
    gmax = min(NQB - 1, jb)
    coff = (gmin - (jb - 2)) * 128
    return gmin, gmax, coff


def _build(loop_iters=0, stages=3, skip_norm=False, skip_mask=False, skip_exp=False):
    nc = bacc.Bacc(None, target_bir_lowering=False, name="swattn")

    xT = nc.dram_tensor("xT", [C, TL], BF16, kind="ExternalInput")
    wqkT = nc.dram_tensor("wqkT", [C, 3 * C], BF16, kind="ExternalInput")
    wpT = nc.dram_tensor("wpT", [C, C], F32R, kind="ExternalInput")
    kb = nc.dram_tensor("kb", [128, NJB], F32, kind="ExternalInput")
    mask = nc.dram_tensor("mask", [128, 2, 384], BF16, kind="ExternalInput")
    y = nc.dram_tensor("y", [TQ, C], F32, kind="ExternalOutput")
    with tile.TileContext(nc) as tc:
        with tc.tile_pool(name="persist", bufs=1) as pers:
            import contextlib
            loop = tc.For_i(0, loop_iters, 1) if loop_iters else contextlib.nullcontext()
            with loop:
                kb_sb = pers.tile([128, NJB], F32)
                mask_sb = pers.tile([128, 2, 384], BF16)
                # per-k-tile input tiles + chunked DMAs so the first matmuls start
                # early (one monolithic DMA stalled the PE ~19us at kernel start)
                xT_r = xT[:].rearrange("(ko ki) t -> ki ko t", ki=128)
                wqk_r = wqkT[:].rearrange("(ko ki) f -> ki ko f", ki=128)
                wp_r = wpT[:].rearrange("(ko ki) f -> ki ko f", ki=128)
                xT_k = [pers.tile([128, TL], BF16, name=f"xT{k}") for k in range(KT)]
                wqk_k = [pers.tile([128, 3 * C], BF16, name=f"wqk{k}") for k in range(KT)]
                wp_k = [pers.tile([128, C], F32R, name=f"wp{k}") for k in range(KT)]
                # x chunks on the sync HWDGE queue, weights on the scalar HWDGE
                # queue -- two descriptor generators run in parallel, and each
                # QKV group unblocks as soon as its k-chunks land.
                if stages >= 1:
                    for k in range(KT):
                        nc.sync.dma_start(xT_k[k][:, :640], xT_r[:, k, :640])
                        nc.scalar.dma_start(
                            wqk_k[k][:, : 2 * C], wqk_r[:, k, : 2 * C]
                        )
                        nc.sync.dma_start(xT_k[k][:, 640:], xT_r[:, k, 640:])
                        nc.scalar.dma_start(
                            wqk_k[k][:, 2 * C :], wqk_r[:, k, 2 * C :]
                        )
                    for k in range(KT):
                        nc.scalar.dma_start(wp_k[k][:], wp_r[:, k, :])
                    nc.sync.dma_start(kb_sb[:], kb[:])
                    nc.sync.dma_start(mask_sb[:], mask[:])

                # Q,K head-major [d, t]; pair p: partitions 0:64 = head 2p, 64:128 = head 2p+1
                qT_sb = pers.tile([128, NPAIR, TQ], BF16)
                kT_sb = pers.tile([128, NPAIR, TL], BF16)
                # V token-major, padded with a 64-wide ones block per head:
                # AV matmuls with lhsT=[V_h | ones] then write yT_un on psum
                # partitions 0:64 and the replicated softmax denominator on
                # partitions 64:128 -- broadcast comes free from the matmul.
                # ones block FIRST (cols 0:D) so the softmax denominator lands
                # on psum partitions 0:64: reciprocal_approx_fast reads PSUM
                # correctly only at partition base 0 (base-64 psum reads are
                # garbage on hardware).
                v_sb = pers.tile([128, NJB, H, 128], BF16)
                nc.gpsimd.memset(v_sb[:, :, :, :D], 1.0)
                # normalized attention output, c-major [c, t]
                yt_sb = pers.tile([128, KT, TQ], F32R)

                if stages < 1:
                    dummy = pers.tile([128, 64], F32, name="dummy")
                    nc.vector.memset(dummy[:], 1.0)
                # ---- stage 1: QKV projection ----
                # ps_a spans stages 1 and 3: the projection reuses the QKV psum
                # banks (same tag), so it never WAR-blocks on attention psum.
                with tc.tile_pool(name="ps_a", bufs=2, space="PSUM") as ps_qkv:
                  if stages >= 1:
                    # Q: own queries only (local tokens 256:1280); two
                    # 512-wide matmul groups share one 2-bank psum tile so a
                    # single wide eviction replaces two.
                    for p in range(NPAIR):
                        pq = ps_qkv.tile([128, TQ], F32, tag="qkv2", name=f"pq{p}")
                        for t0 in range(0, TQ, 512):
                            for k in range(KT):
                                nc.tensor.matmul(
                                    pq[:, t0 : t0 + 512],
                                    wqk_k[k][:, p * 128 : (p + 1) * 128],
                                    xT_k[k][:, MEM + t0 : MEM + t0 + 512],
                                    start=(k == 0), stop=(k == KT - 1),
                                )
                        nc.scalar.copy(qT_sb[:, p, :], pq[:])
                    # K: all local tokens; 1024-wide tile + 256 remainder
                    for p in range(NPAIR):
                        pk = ps_qkv.tile([128, TQ], F32, tag="qkv2", name=f"pk{p}")
                        for t0 in (0, 512):
                            for k in range(KT):
                                nc.tensor.matmul(
                                    pk[:, t0 : t0 + 512],
                                    wqk_k[k][:, C + p * 128 : C + (p + 1) * 128],
                                    xT_k[k][:, t0 : t0 + 512],
                                    start=(k == 0), stop=(k == KT - 1),
                                )
                        nc.vector.tensor_copy(kT_sb[:, p, 0:TQ], pk[:])
                        pk2 = ps_qkv.tile([128, 256], F32, tag="qkv", name=f"pk2{p}")
                        for k in range(KT):
                            nc.tensor.matmul(
                                pk2[:],
                                wqk_k[k][:, C + p * 128 : C + (p + 1) * 128],
                                xT_k[k][:, TQ : TQ + 256],
                                start=(k == 0), stop=(k == KT - 1),
                            )
                        nc.vector.tensor_copy(kT_sb[:, p, TQ:], pk2[:])
                    # V: token-major, two token-blocks per psum tile
                    for tb in range(0, NJB, 2):
                        pv = ps_qkv.tile([128, TQ], F32, tag="qkv2", name=f"pv{tb}")
                        for sub in range(2):
                            for k in range(KT):
                                nc.tensor.matmul(
                                    pv[:, sub * 512 : (sub + 1) * 512],
                                    xT_k[k][:, (tb + sub) * 128 : (tb + sub + 1) * 128],
                                    wqk_k[k][:, 2 * C : 3 * C],
                                    start=(k == 0), stop=(k == KT - 1),
                                )
                        nc.scalar.copy(
                            v_sb[:, tb : tb + 2, :, D:],
                            pv[:].rearrange("t (b h d) -> t b h d", b=2, h=H),
                        )

                # ---- stage 2: attention, one head pair at a time ----
                if stages < 2:
                    pass
                elif True:
                  with (
                    tc.tile_pool(name="ps_s", bufs=2, space="PSUM") as ps_s,
                    tc.tile_pool(name="ps_y", bufs=2, space="PSUM") as ps_y,
                    tc.tile_pool(name="ptile", bufs=4) as ppool,
                    tc.tile_pool(name="norm", bufs=3) as npool,
                ):
                    for p in range(NPAIR):
                        yps = [
                            ps_y.tile([128, TQ], F32, tag="yt", name=f"yt{p}_{i}")
                            for i in range(2)
                        ]
                        def emit_s(jb):
                            gmin, gmax, coff = _consumers(jb)
                            ncols = (gmax - gmin + 1) * 128
                            s_ps = ps_s.tile([128, 2, 512], F32, tag="s", name=f"s{p}_{jb}")
                            for hh in range(2):
                                nc.tensor.matmul(
                                    s_ps[:, hh, :ncols],
                                    kT_sb[hh * 64 : hh * 64 + 64, p, jb * 128 : (jb + 1) * 128],
                                    qT_sb[hh * 64 : hh * 64 + 64, p, gmin * 128 : (gmax + 1) * 128],
                                    start=True, stop=True,
                                )
                            return s_ps

                        def emit_rest(jb, s_ps):
                            gmin, gmax, coff = _consumers(jb)
                            ncols = (gmax - gmin + 1) * 128
                            p_sb = ppool.tile([128, 2, 384], BF16, tag="p", name=f"p{p}_{jb}")
                            if skip_exp:
                                nc.vector.tensor_copy(p_sb[:, :, :ncols], s_ps[:, :, :ncols])
                            else:
                                nc.scalar.activation(
                                    p_sb[:, :, :ncols],
                                    s_ps[:, :, :ncols],
                                    mybir.ActivationFunctionType.Exp,
                                    bias=kb_sb[:, jb : jb + 1],
                                    scale=0.125,
                                )
                            # only the two triangular 128-col blocks of the
                            # band need masking; the middle block is all-ones
                            mranges = [
                                r0 for r0 in range(0, ncols, 128)
                                if coff + r0 in (0, 256)
                            ]
                            if skip_mask:
                                mranges = []
                            if mranges == [0, 256]:
                                # one strided op covering both triangle blocks
                                nc.vector.tensor_tensor(
                                    p_sb[:, :, :].rearrange(
                                        "p h (r c) -> p h r c", c=128
                                    )[:, :, 0:3:2],
                                    p_sb[:, :, :].rearrange(
                                        "p h (r c) -> p h r c", c=128
                                    )[:, :, 0:3:2],
                                    mask_sb[:, :, :].rearrange(
                                        "p h (r c) -> p h r c", c=128
                                    )[:, :, 0:3:2],
                                    mybir.AluOpType.mult,
                                )
                            else:
                                for r0 in mranges:
                                    nc.vector.tensor_tensor(
                                        p_sb[:, :, r0 : r0 + 128],
                                        p_sb[:, :, r0 : r0 + 128],
                                        mask_sb[:, :, coff + r0 : coff + r0 + 128],
                                        mybir.AluOpType.mult,
                                    )

                            # AV: one wide matmul per (head, key-block), split at
                            # the 512-col PSUM bank boundary. All start=False --
                            # the banks were zero-cleared by the K=1 matmuls above
                            # (start=True clears has_written for the WHOLE bank,
                            # so per-column-group starts are unusable).
                            c0 = gmin * 128
                            c1 = (gmax + 1) * 128
                            for hh in range(2):
                                h = 2 * p + hh
                                for a, b in ((c0, min(c1, 512)), (max(c0, 512), c1)):
                                    if a >= b:
                                        continue
                                    # start=True exactly on the first
                                    # sub-matmul touching each 512-col bank:
                                    # it clears the whole bank's has_written,
                                    # so later matmuls overwrite-and-set
                                    # fresh columns and accumulate written
                                    # ones -- no explicit zero-fill needed.
                                    nc.tensor.matmul(
                                        yps[hh][:, a:b],
                                        v_sb[:, jb, h, :],
                                        p_sb[:, hh, a - c0 : b - c0],
                                        start=(jb == 0 and a == 0)
                                        or (jb == 4 and a == 512),
                                        stop=(jb == NJB - 1 and b == c1),
                                        skip_group_check=True,
                                    )

                            # normalization once per head after the last key-block:
                            # denominator is replicated on psum partitions 64:128, so
                            # it is one DVE reciprocal (base-shift 64->0) and one
                            # psum*sbuf multiply over the full 1024 columns.
                            if jb == NJB - 1 and not skip_norm:
                                with nc.allow_low_precision(
                                    reason="softmax weights are O(1); bf16 out is ample"
                                ):
                                    for hh in range(2):
                                        # den is on psum partitions 0:64 (ones
                                        # block first); reciprocal_approx_fast
                                        # reads psum correctly only at base 0.
                                        rec = npool.tile([64, TQ], F32, tag="rec")
                                        nc.vector.reciprocal_approx_fast(
                                            rec[:], yps[hh][0:64, :]
                                        )
                                        nc.vector.tensor_tensor(
                                            yt_sb[hh * 64 : hh * 64 + 64, p, :],
                                            yps[hh][64:128, :],
                                            rec[:],
                                            mybir.AluOpType.mult,
                                        )

                    # 2-stage software pipeline: the PE stream must carry
                        # S(jb+1) BEFORE AV(jb), since engines execute their
                        # streams strictly in order -- otherwise AV(jb) stalling
                        # on exp/mask(jb) blocks the already-ready S(jb+1).
                        pending = None
                        for jb in range(NJB):
                            sp = emit_s(jb)
                            if pending is not None:
                                emit_rest(pending[0], pending[1])
                            pending = (jb, sp)
                        emit_rest(pending[0], pending[1])

                # ---- stage 3: output projection ----
                if stages < 3:
                    pass
                elif True:
                  with (
                    tc.tile_pool(name="ps_o", bufs=4, space="PSUM") as ps_o,
                    tc.tile_pool(name="obuf", bufs=4) as opool,
                ):
                    for g in range(NQB):
                        po = ps_o.tile([128, C], F32, tag="o")
                        for k in range(KT):
                            nc.tensor.matmul(
                                po[:],
                                yt_sb[:, k, g * 128 : (g + 1) * 128],
                                wp_k[k][:],
                                start=(k == 0), stop=(k == KT - 1),
                            )
                        o_sb = opool.tile([128, C], F32, tag="ob")
                        nc.scalar.copy(o_sb[:], po[:])
                        nc.sync.dma_start(y[g * 128 : (g + 1) * 128, :], o_sb[:])

    nc.finalize()
    return nc


def _host_inputs(x, w_attn, w_proj):
    """Build per-core input maps (numpy only)."""
    wqkT = np.ascontiguousarray(w_attn.T.astype(ml_dtypes.bfloat16))
    wpT = np.ascontiguousarray(w_proj.T.astype(np.float32))

    # band mask [128, 384]: valid iff 0 <= c - b <= MEM
    b = np.arange(128)[:, None]
    c = np.arange(384)[None, :]
    mask = ((c - b >= 0) & (c - b <= MEM)).astype(ml_dtypes.bfloat16)
    mask = np.ascontiguousarray(np.broadcast_to(mask[:, None, :], (128, 2, 384)))

    in_maps = []
    for core in range(NCORES):
        bi, ci = divmod(core, T // TQ)
        q0 = ci * TQ
        x_loc = np.zeros((TL, C), dtype=np.float32)
        lo = q0 - MEM
        src0 = max(0, lo)
        x_loc[src0 - lo :] = x[bi, src0 : q0 + TQ]
        xT_loc = np.ascontiguousarray(x_loc.T.astype(ml_dtypes.bfloat16))

        kb = np.zeros((128, NJB), dtype=np.float32)
        if lo < 0:
            pad = -lo  # number of padded (invalid) leading keys
            for jb in range(NJB):
                k0 = jb * 128
                if k0 >= pad:
                    break
                kb[: min(128, pad - k0), jb] = MASKVAL

        in_maps.append(
            {"xT": xT_loc, "wqkT": wqkT, "wpT": wpT, "kb": kb, "mask": mask}
        )
    return in_maps


def kernel(x, w_attn, w_proj):
    x = np.asarray(x, dtype=np.float32)
    w_attn = np.asarray(w_attn, dtype=np.float32)
    w_proj = np.asarray(w_proj, dtype=np.float32)

    if "nc" not in _cache:
        _cache["nc"] = _build()
    nc = _cache["nc"]

    in_maps = _host_inputs(x, w_attn, w_proj)
    res = run_bass_kernel_spmd(nc, in_maps, core_ids=list(range(NCORES)))

    out = np.empty((B, T, C), dtype=np.float32)
    for core in range(NCORES):
        bi, ci = divmod(core, T // TQ)
        out[bi, ci * TQ : (ci + 1) * TQ] = res.results[core]["y"]
    return out

